# revision 7
# baseline (speedup 1.0000x reference)
"""ECE loss kernel for Trainium2, data-parallel over 8 NeuronCores.

Math: the reference ECE reduces exactly to

    ece = (1/n) * sum_b | D_b |,   D_b = sum_{i: bin_i = b} (p_i - acc_i)

since (count/n)*|sum_conf - sum_acc|/count == |sum_conf - sum_acc|/n and
empty bins contribute 0.  Per element only d_i = p_i - acc_i and the bin of
p_i matter.  Bin membership (bin <= k) is evaluated directly as
p <= float32((k+1)/10), so no bin-index tensor is materialized.  The 9
cumulative masked sums S_k = sum d * (p <= t_k) are one scalar_tensor_tensor
instruction each (compare + multiply + free-axis accumulate); the host
differences them into per-bin sums.

Each core processes a contiguous 2^21-element shard laid out [128, 16384] in
a single chunk (instruction count dominates cost in this deployment: ~50 us
per compute instruction, so the kernel is 12 compute instructions).  Device
output per core: [128, 10] fp32 partials (S_0..S_8, T).  Host: sum partials
over partitions/cores, difference, abs, normalize.
"""

import numpy as np
import ml_dtypes
from contextlib import ExitStack

N_BINS = 10
BATCH = 16_777_216
N_CORES = 8
P = 128
PER_CORE = BATCH // N_CORES            # 2_097_152
FREE = PER_CORE // P                   # 16384
STATS_COLS = 10                        # S_0..S_8, T
N_GPSIMD_MASKS = 0                     # trailing masked sums offloaded to GPSIMD

_NC = None
LAST_RESULTS = None


def _build_nc(repeats: int = 1, gpsimd_masks: int = N_GPSIMD_MASKS):
    import concourse.tile as tile
    from concourse import bacc, mybir

    nc = bacc.Bacc("TRN2", target_bir_lowering=False, debug=False)

    x_d = nc.dram_tensor("logits", [P, FREE], mybir.dt.float32, kind="ExternalInput")
    lab_d = nc.dram_tensor("labels", [P, FREE], mybir.dt.bfloat16, kind="ExternalInput")
    stats_d = nc.dram_tensor(
        "stats", [P, STATS_COLS], mybir.dt.float32, kind="ExternalOutput"
    )

    A = mybir.AluOpType

    with tile.TileContext(nc) as tc, ExitStack() as ctx:
        pool = ctx.enter_context(tc.tile_pool(name="main", bufs=1))

        stats = pool.tile([P, STATS_COLS], mybir.dt.float32)

        for _ in range(repeats):
            x_t = pool.tile([P, FREE], mybir.dt.float32, tag="x")
            nc.sync.dma_start(x_t[:], x_d.ap())
            lab_t = pool.tile([P, FREE], mybir.dt.bfloat16, tag="lab")
            nc.sync.dma_start(lab_t[:], lab_d.ap())

            # p = sigmoid(x), in place (x is dead afterwards)
            nc.scalar.activation(
                x_t[:], x_t[:], mybir.ActivationFunctionType.Sigmoid
            )

            # acc = ((p > 0.5) == lab), in place over lab
            nc.vector.scalar_tensor_tensor(
                lab_t[:], x_t[:], 0.5, lab_t[:], A.is_gt, A.is_equal
            )

            # d = p - acc, free-axis accumulate -> T
            d_t = pool.tile([P, FREE], mybir.dt.bfloat16, tag="d")
            nc.vector.scalar_tensor_tensor(
                d_t[:], x_t[:], 0.0, lab_t[:], A.add, A.subtract,
                accum_out=stats[:, 9:10],
            )

            # S_k = sum d * (p <= (k+1)/10); outputs are dead, aliased over
            # the lab tile (acc is dead after d)
            scr = lab_t[:]
            for k in range(9):
                eng = nc.gpsimd if k >= 9 - gpsimd_masks else nc.vector
                eng.scalar_tensor_tensor(
                    scr, x_t[:], float(np.float32((k + 1) / 10.0)), d_t[:],
                    A.is_le, A.mult,
                    accum_out=stats[:, k : k + 1],
                )

        nc.sync.dma_start(stats_d.ap(), stats[:])

    nc.compile()
    return nc


def _get_nc():
    global _NC
    if _NC is None:
        _NC = _build_nc()
    return _NC


def kernel(logits: np.ndarray, labels: np.ndarray) -> np.ndarray:
    global LAST_RESULTS
    from concourse.bass_utils import run_bass_kernel_spmd

    nc = _get_nc()

    lg = np.ascontiguousarray(np.asarray(logits, dtype=np.float32)).reshape(
        N_CORES, P, FREE
    )
    lb = (
        np.ascontiguousarray(np.asarray(labels, dtype=np.float32))
        .astype(ml_dtypes.bfloat16)
        .reshape(N_CORES, P, FREE)
    )

    in_maps = [{"logits": lg[c], "labels": lb[c]} for c in range(N_CORES)]
    res = run_bass_kernel_spmd(nc, in_maps, core_ids=list(range(N_CORES)))
    LAST_RESULTS = res

    S = np.zeros(STATS_COLS, np.float64)
    for c in range(N_CORES):
        S += res.results[c]["stats"].astype(np.float64).sum(axis=0)

    Sk, T = S[:9], S[9]
    D = np.empty(10, np.float64)
    D[0] = Sk[0]
    D[1:9] = Sk[1:9] - Sk[0:8]
    D[9] = T - Sk[8]
    ece = np.abs(D).sum() / BATCH
    return np.array([ece], dtype=np.float32)


# revision 8
# speedup vs baseline: 1.0029x; 1.0029x over previous
"""ECE loss kernel for Trainium2, data-parallel over 8 NeuronCores.

Math: the reference ECE reduces exactly to

    ece = (1/n) * sum_b | D_b |,   D_b = sum_{i: bin_i = b} (p_i - acc_i)

since (count/n)*|sum_conf - sum_acc|/count == |sum_conf - sum_acc|/n and
empty bins contribute 0.  Per element only d_i = p_i - acc_i and the bin of
p_i matter.  Bin membership (bin <= k) is evaluated directly as
p <= float32((k+1)/10), so no bin-index tensor is materialized.  The 9
cumulative masked sums S_k = sum d * (p <= t_k) are one scalar_tensor_tensor
instruction each (compare + multiply + free-axis accumulate); the host
differences them into per-bin sums.

Each core processes a contiguous 2^21-element shard laid out [128, 16384] in
a single chunk (instruction count dominates cost in this deployment: ~50 us
per compute instruction, so the kernel is 12 compute instructions).  Device
output per core: [128, 10] fp32 partials (S_0..S_8, T).  Host: sum partials
over partitions/cores, difference, abs, normalize.
"""

import numpy as np
import ml_dtypes
from contextlib import ExitStack

N_BINS = 10
BATCH = 16_777_216
N_CORES = 8
P = 128
PER_CORE = BATCH // N_CORES            # 2_097_152
FREE = PER_CORE // P                   # 16384
STATS_COLS = 10                        # S_0..S_8, T
N_GPSIMD_MASKS = 0                     # trailing masked sums offloaded to GPSIMD

_NC = None
LAST_RESULTS = None


def _build_nc(repeats: int = 1, gpsimd_masks: int = N_GPSIMD_MASKS):
    import concourse.tile as tile
    from concourse import bacc, mybir

    nc = bacc.Bacc("TRN2", target_bir_lowering=False, debug=False)

    x_d = nc.dram_tensor("logits", [P, FREE], mybir.dt.float32, kind="ExternalInput")
    lab_d = nc.dram_tensor("labels", [P, FREE], mybir.dt.bfloat16, kind="ExternalInput")
    stats_d = nc.dram_tensor(
        "stats", [P, STATS_COLS], mybir.dt.float32, kind="ExternalOutput"
    )

    A = mybir.AluOpType

    with tile.TileContext(nc) as tc, ExitStack() as ctx:
        pool = ctx.enter_context(tc.tile_pool(name="main", bufs=1))

        stats = pool.tile([P, STATS_COLS], mybir.dt.float32)

        for _ in range(repeats):
            x_t = pool.tile([P, FREE], mybir.dt.float32, tag="x")
            nc.sync.dma_start(x_t[:], x_d.ap())
            lab_t = pool.tile([P, FREE], mybir.dt.bfloat16, tag="lab")
            nc.sync.dma_start(lab_t[:], lab_d.ap())

            # p = sigmoid(x), in place (x is dead afterwards)
            nc.scalar.activation(
                x_t[:], x_t[:], mybir.ActivationFunctionType.Sigmoid
            )

            # acc = ((p > 0.5) == lab), in place over lab
            nc.vector.scalar_tensor_tensor(
                lab_t[:], x_t[:], 0.5, lab_t[:], A.is_gt, A.is_equal
            )

            # d = p - acc, free-axis accumulate -> T
            d_t = pool.tile([P, FREE], mybir.dt.bfloat16, tag="d")
            nc.vector.scalar_tensor_tensor(
                d_t[:], x_t[:], 0.0, lab_t[:], A.add, A.subtract,
                accum_out=stats[:, 9:10],
            )

            # S_k = sum d * (p <= (k+1)/10); outputs are dead, aliased over
            # the lab tile (acc is dead after d)
            scr = lab_t[:]
            for k in range(9):
                eng = nc.gpsimd if k >= 9 - gpsimd_masks else nc.vector
                eng.scalar_tensor_tensor(
                    scr, x_t[:], float(np.float32((k + 1) / 10.0)), d_t[:],
                    A.is_le, A.mult,
                    accum_out=stats[:, k : k + 1],
                )

        nc.sync.dma_start(stats_d.ap(), stats[:])

    nc.compile()
    return nc


def _get_nc():
    global _NC
    if _NC is None:
        _NC = _build_nc()
    return _NC


def kernel(logits: np.ndarray, labels: np.ndarray) -> np.ndarray:
    global LAST_RESULTS
    from concourse.bass_utils import run_bass_kernel_spmd

    nc = _get_nc()

    lg = np.ascontiguousarray(np.asarray(logits, dtype=np.float32)).reshape(
        N_CORES, P, FREE
    )
    lb = (
        np.ascontiguousarray(np.asarray(labels, dtype=np.float32))
        .astype(ml_dtypes.bfloat16)
        .reshape(N_CORES, P, FREE)
    )

    in_maps = [{"logits": lg[c], "labels": lb[c]} for c in range(N_CORES)]
    try:
        res = run_bass_kernel_spmd(nc, in_maps, core_ids=list(range(N_CORES)))
    except Exception:
        # A prior tenant can leave the shared device unrecoverable; a fresh
        # PJRT backend usually restores it.  Best-effort single retry.
        import jax

        try:
            from jax.extend.backend import clear_backends

            clear_backends()
        except Exception:
            pass
        jax.clear_caches()
        res = run_bass_kernel_spmd(nc, in_maps, core_ids=list(range(N_CORES)))
    LAST_RESULTS = res

    S = np.zeros(STATS_COLS, np.float64)
    for c in range(N_CORES):
        S += res.results[c]["stats"].astype(np.float64).sum(axis=0)

    Sk, T = S[:9], S[9]
    D = np.empty(10, np.float64)
    D[0] = Sk[0]
    D[1:9] = Sk[1:9] - Sk[0:8]
    D[9] = T - Sk[8]
    ece = np.abs(D).sum() / BATCH
    return np.array([ece], dtype=np.float32)


# revision 9
# speedup vs baseline: 1.0896x; 1.0864x over previous
"""ECE loss kernel for Trainium2, data-parallel over 8 NeuronCores.

Math: the reference ECE reduces exactly to

    ece = (1/n) * sum_b | D_b |,   D_b = sum_{i: bin_i = b} (p_i - acc_i)

since (count/n)*|sum_conf - sum_acc|/count == |sum_conf - sum_acc|/n and
empty bins contribute 0.  Per element only d_i = p_i - acc_i and the bin of
p_i matter.  The bin index is materialized once as int16(10*p - 0.5) (the
DVE float->int output convert rounds to nearest on HW, giving ceil(10p)-1
except where 10p is an exact fp32 integer - a measure-zero set here), so the
9 cumulative masked sums S_k = sum d * (bin <= k) run with 16-bit operands
in the DVE 2x perf mode.  Each S_k is a single scalar_tensor_tensor
instruction (compare + multiply + free-axis accumulate); the host
differences them into per-bin sums.

Each core processes a contiguous 2^21-element shard laid out [128, 16384] in
a single chunk: 13 compute instructions + 3 DMAs (per-instruction overhead
dominates cost in this deployment).  Device output per core: [128, 10] fp32
partials (S_0..S_8 and T = sum d).  Host: sum partials over partitions and
cores, difference, abs, normalize.
"""

import numpy as np
import ml_dtypes
from contextlib import ExitStack

N_BINS = 10
BATCH = 16_777_216
N_CORES = 8
P = 128
PER_CORE = BATCH // N_CORES            # 2_097_152
FREE = PER_CORE // P                   # 16384
STATS_COLS = 10                        # S_0..S_8, T

_NC = None
LAST_RESULTS = None


def _build_nc(repeats: int = 1):
    import concourse.tile as tile
    from concourse import bacc, mybir

    nc = bacc.Bacc("TRN2", target_bir_lowering=False, debug=False)

    x_d = nc.dram_tensor("logits", [P, FREE], mybir.dt.float32, kind="ExternalInput")
    lab_d = nc.dram_tensor("labels", [P, FREE], mybir.dt.bfloat16, kind="ExternalInput")
    stats_d = nc.dram_tensor(
        "stats", [P, STATS_COLS], mybir.dt.float32, kind="ExternalOutput"
    )

    A = mybir.AluOpType

    with tile.TileContext(nc) as tc, ExitStack() as ctx:
        pool = ctx.enter_context(tc.tile_pool(name="main", bufs=1))

        stats = pool.tile([P, STATS_COLS], mybir.dt.float32)

        for _ in range(repeats):
            x_t = pool.tile([P, FREE], mybir.dt.float32, tag="x")
            nc.sync.dma_start(x_t[:], x_d.ap())
            lab_t = pool.tile([P, FREE], mybir.dt.bfloat16, tag="lab")
            nc.sync.dma_start(lab_t[:], lab_d.ap())

            # p = sigmoid(x), in place (x is dead afterwards)
            nc.scalar.activation(
                x_t[:], x_t[:], mybir.ActivationFunctionType.Sigmoid
            )

            # bin = int16(10p - 0.5): HW float->int convert rounds to nearest
            binf = pool.tile([P, FREE], mybir.dt.int16, tag="bin")
            nc.vector.tensor_scalar(
                binf[:], x_t[:], 10.0, 0.5, A.mult, A.subtract
            )

            # acc = ((bin >= 5) == lab), in place over lab
            nc.vector.scalar_tensor_tensor(
                lab_t[:], binf[:], 4.5, lab_t[:], A.is_ge, A.is_equal
            )

            # d = p - acc, free-axis accumulate -> T
            d_t = pool.tile([P, FREE], mybir.dt.bfloat16, tag="d")
            nc.vector.scalar_tensor_tensor(
                d_t[:], x_t[:], 0.0, lab_t[:], A.add, A.subtract,
                accum_out=stats[:, 9:10],
            )

            # S_k = sum d * (bin <= k); the full-size output is dead, aliased
            # over the lab tile (acc is dead after d)
            scr = lab_t[:]
            for k in range(9):
                nc.vector.scalar_tensor_tensor(
                    scr, binf[:], k + 0.5, d_t[:], A.is_le, A.mult,
                    accum_out=stats[:, k : k + 1],
                )

        nc.sync.dma_start(stats_d.ap(), stats[:])

    nc.compile()
    return nc


def _get_nc():
    global _NC
    if _NC is None:
        _NC = _build_nc()
    return _NC


def kernel(logits: np.ndarray, labels: np.ndarray) -> np.ndarray:
    global LAST_RESULTS
    from concourse.bass_utils import run_bass_kernel_spmd

    nc = _get_nc()

    lg = np.ascontiguousarray(np.asarray(logits, dtype=np.float32)).reshape(
        N_CORES, P, FREE
    )
    lb = (
        np.ascontiguousarray(np.asarray(labels, dtype=np.float32))
        .astype(ml_dtypes.bfloat16)
        .reshape(N_CORES, P, FREE)
    )

    in_maps = [{"logits": lg[c], "labels": lb[c]} for c in range(N_CORES)]
    try:
        res = run_bass_kernel_spmd(nc, in_maps, core_ids=list(range(N_CORES)))
    except Exception:
        # A prior tenant can leave the shared device unrecoverable; a fresh
        # PJRT backend usually restores it.  Best-effort single retry.
        import jax

        try:
            from jax.extend.backend import clear_backends

            clear_backends()
        except Exception:
            pass
        jax.clear_caches()
        res = run_bass_kernel_spmd(nc, in_maps, core_ids=list(range(N_CORES)))
    LAST_RESULTS = res

    S = np.zeros(STATS_COLS, np.float64)
    for c in range(N_CORES):
        S += res.results[c]["stats"].astype(np.float64).sum(axis=0)

    Sk, T = S[:9], S[9]
    D = np.empty(10, np.float64)
    D[0] = Sk[0]
    D[1:9] = Sk[1:9] - Sk[0:8]
    D[9] = T - Sk[8]
    ece = np.abs(D).sum() / BATCH
    return np.array([ece], dtype=np.float32)
